# revision 24
# baseline (speedup 1.0000x reference)
"""Day-routed adapter MLP (per-sample day-specific 2-layer MLP + LayerNorm)
for 8 Trainium2 NeuronCores.

Computation per sample b (day d = day_indices[b]):
    h = relu(x[b] @ W1[d] + b1[d])        # [T, D_hid]
    y = h @ W2[d] + b2[d]                 # [T, D_out]
    out = LN(y) * gamma[d] + beta[d]      # LN over last dim

Sharding: data-parallel over batch, 8 samples per core. The per-sample
day weights are gathered on the host (routing is host-visible), and x is
pre-transposed on the host so the device needs no transposes at all:

  pass 1:  hT[h_chunk, :T] += W1[k_chunk, h_chunk].T @ xT[k_chunk, :T]
           (lhsT = W1 natural layout, rhs = xT)  -> hT with H on partitions,
           so b1 is a per-partition bias fused into the ReLU copyback (ACT).
  pass 2:  y[t_tile, :O]  += hT[k_chunk, t_tile].T @ W2[k_chunk, :O]
           (lhsT = hT from pass 1, rhs = W2 natural layout) -> y with T on
           partitions and O on the free axis, which is exactly the layout
           LayerNorm wants (bn_stats/bn_aggr reduce along free axis).
"""

import os

import numpy as np
import ml_dtypes

import concourse.bass as bass
import concourse.mybir as mybir
import concourse.tile as tile
from concourse import bacc
from concourse.bass_utils import run_bass_kernel_spmd

N_CORES = 8
B, T, D_IN = 64, 512, 512
D_HID, D_OUT = 1024, 512
S = B // N_CORES  # samples per core
EPS = 1e-5

P = 128
KD = D_IN // P   # 4 contraction chunks in pass 1
KH = D_HID // P  # 8 contraction chunks in pass 2 (= H chunks of pass 1 out)
MT = T // P      # 4 token tiles in pass 2

# Matmul input dtype. float16: full PE rate (1 cyc/row, FWL hides weight
# loads), half the DMA bytes of fp32, and a 10-bit mantissa (~4x better than
# bf16; fp32 accumulate in PSUM). float32r: fp32 storage but ~2 cyc/row and
# 2x the DMA traffic. bfloat16: same speed as fp16, worse precision.
MM_DTYPE = os.environ.get("DAYMLP_MM_DTYPE", "float16")

_cache: dict = {}
last_run_result = None  # stash of BassKernelResults for test harness use


def _build(mm_dtype_name: str, apply_affine: bool) -> bass.Bass:
    f32 = mybir.dt.float32
    # DRAM inputs and the SBUF tiles feeding the PE carry the matmul dtype
    # directly (for float32r the producing writes perform the required
    # rounding; fp16/bf16 arrays are cast host-side).
    store_dt = getattr(mybir.dt, mm_dtype_name)
    dram_dt = store_dt

    # Bacc (not raw Bass): its compile pipeline moves extra matmul waits onto
    # ldweights and splits >1-wait instructions via event semaphores, which
    # the TRN2 ISA requires.
    nc = bacc.Bacc("TRN2", target_bir_lowering=False)
    # Partition-major DRAM layouts: each SBUF partition's data is one
    # contiguous DRAM run, so every load is 128 large descriptors instead of
    # 128*K small ones (the DMA engines are descriptor-rate limited).
    xt_d = nc.dram_tensor("xt", [S, P, KD, T], dram_dt, kind="ExternalInput")
    w1_d = nc.dram_tensor("w1", [S, P, 2, KD, D_HID // 2], dram_dt, kind="ExternalInput")
    b1_d = nc.dram_tensor("b1", [S, P, KH], f32, kind="ExternalInput")
    w2_d = nc.dram_tensor("w2", [S, P, KH, D_OUT], dram_dt, kind="ExternalInput")
    b2_d = nc.dram_tensor("b2", [S, D_OUT], f32, kind="ExternalInput")
    if apply_affine:
        gm_d = nc.dram_tensor("gm", [S, D_OUT], f32, kind="ExternalInput")
        bt_d = nc.dram_tensor("bt", [S, D_OUT], f32, kind="ExternalInput")
    y_d = nc.dram_tensor("y", [S, T, D_OUT], store_dt, kind="ExternalOutput")

    with tile.TileContext(nc) as tc:
        with (
            tc.tile_pool(name="xw", bufs=2) as xw,
            tc.tile_pool(name="hb", bufs=2) as hb,
            tc.tile_pool(name="vec", bufs=2) as vec,
            tc.tile_pool(name="yp", bufs=6) as yp,
            tc.tile_pool(name="st", bufs=8) as st,
            tc.tile_pool(name="consts", bufs=1) as cpool,
            tc.tile_pool(name="prologue", bufs=1) as pro,
            tc.tile_pool(name="psum", bufs=8, space="PSUM") as pp,
        ):
            eps_t = cpool.tile([P, 1], f32)
            nc.vector.memset(eps_t, EPS)

            # PE pre-warm: matmuls on a zeroed tile while the first real
            # operands are still in flight. The PE clock-gate (HAM) needs
            # ~3.4us of sustained activity to reach 2.4GHz; warming during
            # the DMA head means the real matmuls run at full rate. N=128
            # warm matmuls give fine-grained control over when the PE frees
            # up for the first data-dependent matmul (a too-long warm block
            # delays the real stream past the DMA arrival). The warm matmuls
            # write zeros into sample 0's real accumulation banks
            # (start=True) and sample 0's k=0 matmuls accumulate on top —
            # this keeps them live (walrus dead-code-eliminates matmuls
            # whose PSUM output is never read).
            n_warm = int(os.environ.get("DAYMLP_WARM_MMS", "30"))
            warm_t = cpool.tile([P, P], store_dt, name="warm_t")
            nc.vector.memset(warm_t, 0.0)
            warm_tiles = [
                pp.tile([P, T], f32, tag="ps", name=f"warm_ps_{w}")
                for w in range(min(8, n_warm))
            ]
            for w in range(n_warm):
                nc.tensor.matmul(
                    warm_tiles[w % len(warm_tiles)][:, :P],
                    lhsT=warm_t,
                    rhs=warm_t,
                    start=True,
                    stop=True,
                )

            HH = D_HID // 2
            for s in range(S):
                if s == 0:
                    # prologue: few, k-granular DMAs. The sync sequencer
                    # issues DMA_DIRECT2D at ~0.6us each, so piece count is
                    # the real head cost; but per-k semaphores let the
                    # k-outer matmuls start as soon as chunk k=0 lands.
                    xt0_c = pro.tile([P, T], store_dt, tag="xt0_0", name="xt0_0")
                    nc.sync.dma_start(out=xt0_c, in_=xt_d[s, :, 0, :])
                    w10_c = pro.tile([P, 2, HH], store_dt, tag="w10_0", name="w10_0")
                    nc.sync.dma_start(out=w10_c, in_=w1_d[s, :, :, 0, :])
                    xt1_c = pro.tile([P, T], store_dt, tag="xt0_1", name="xt0_1")
                    nc.sync.dma_start(out=xt1_c, in_=xt_d[s, :, 1, :])
                    w11_c = pro.tile([P, 2, HH], store_dt, tag="w10_1", name="w10_1")
                    nc.sync.dma_start(out=w11_c, in_=w1_d[s, :, :, 1, :])
                    xt23_c, w123_c = [], []
                    for k in range(2, KD):
                        xk = pro.tile([P, T], store_dt, tag=f"xt0_{k}", name=f"xt0_{k}")
                        nc.sync.dma_start(out=xk, in_=xt_d[s, :, k, :])
                        wk = pro.tile([P, 2, HH], store_dt, tag=f"w10_{k}", name=f"w10_{k}")
                        nc.sync.dma_start(out=wk, in_=w1_d[s, :, :, k, :])
                        xt23_c.append(xk)
                        w123_c.append(wk)
                    b1_t = vec.tile([P, KH], f32, tag="b1")
                    nc.sync.dma_start(out=b1_t, in_=b1_d[s])
                    xt_ck = [xt0_c, xt1_c] + xt23_c
                    w1_cks = [w10_c, w11_c] + w123_c

                    def w1_sl(k, h):
                        half, hh = h // (KH // 2), h % (KH // 2)
                        return w1_cks[k][:, half, P * hh : P * (hh + 1)]
                else:
                    b1_t = vec.tile([P, KH], f32, tag="b1")
                    nc.sync.dma_start(out=b1_t, in_=b1_d[s])
                    xt_t = xw.tile([P, KD, T], store_dt, tag="xt")
                    nc.sync.dma_start(out=xt_t, in_=xt_d[s])
                    # w1 in h-halves: pass 1 h0-3 only gates on the first
                    # half, smoothing the early-delivery cliff
                    w1a_t = xw.tile([P, KD, HH], store_dt, tag="w1a")
                    nc.sync.dma_start(out=w1a_t, in_=w1_d[s, :, 0])
                    w1b_t = xw.tile([P, KD, HH], store_dt, tag="w1b")
                    nc.sync.dma_start(out=w1b_t, in_=w1_d[s, :, 1])

                def load_rest(s=s):
                    if s == 0:
                        qs = []
                        for q in range(4):
                            w2q = xw.tile([P, 2, D_OUT], store_dt, tag=f"w2q{q}")
                            nc.sync.dma_start(out=w2q, in_=w2_d[s, :, 2 * q : 2 * q + 2])
                            qs.append(w2q)
                        w2_t = tuple(qs)
                    else:
                        w2_t = xw.tile([P, KH, D_OUT], store_dt, tag="w2")
                        nc.sync.dma_start(out=w2_t, in_=w2_d[s])
                    b2_t = vec.tile([P, 1, D_OUT], f32, tag="b2")
                    nc.sync.dma_start(
                        out=b2_t, in_=b2_d[s : s + 1, :].partition_broadcast(P)
                    )
                    gm_t = bt_t = None
                    if apply_affine:
                        gm_t = vec.tile([P, 1, D_OUT], f32, tag="gm")
                        nc.sync.dma_start(
                            out=gm_t, in_=gm_d[s : s + 1, :].partition_broadcast(P)
                        )
                        bt_t = vec.tile([P, 1, D_OUT], f32, tag="bt")
                        nc.sync.dma_start(
                            out=bt_t, in_=bt_d[s : s + 1, :].partition_broadcast(P)
                        )
                    return w2_t, b2_t, gm_t, bt_t

                if s > 0:
                    # pass-2 operands up front so DMA overlaps pass-1 compute
                    w2_t, b2_t, gm_t, bt_t = load_rest()

                # pass 1: hT[h, :] = relu(W1[:, h].T @ xT + b1[h])
                hT_t = hb.tile([P, KH, T], store_dt, tag="hT")
                if s == 0:
                    # k-outer over all 8 PSUM banks: matmuls start as soon as
                    # chunk k=0 has landed
                    ps_list = [pp.tile([P, T], f32, tag="ps", name=f"ps0_{h}") for h in range(KH)]
                    for k in range(KD):
                        for h in range(KH):
                            nc.tensor.matmul(
                                ps_list[h],
                                lhsT=w1_sl(k, h),
                                rhs=xt_ck[k],
                                start=(k == 0),
                                stop=(k == KD - 1),
                            )
                    w2_t, b2_t, gm_t, bt_t = load_rest()
                    for h in range(KH):
                        nc.scalar.activation(
                            out=hT_t[:, h, :],
                            in_=ps_list[h],
                            func=mybir.ActivationFunctionType.Relu,
                            bias=b1_t[:, h : h + 1],
                            scale=1.0,
                        )
                else:
                    for h in range(KH):
                        w1h_t = w1a_t if h < KH // 2 else w1b_t
                        hh = h % (KH // 2)
                        ps = pp.tile([P, T], f32, tag="ps")
                        for k in range(KD):
                            nc.tensor.matmul(
                                ps,
                                lhsT=w1h_t[:, k, P * hh : P * (hh + 1)],
                                rhs=xt_t[:, k, :],
                                start=(k == 0),
                                stop=(k == KD - 1),
                            )
                        nc.scalar.activation(
                            out=hT_t[:, h, :],
                            in_=ps,
                            func=mybir.ActivationFunctionType.Relu,
                            bias=b1_t[:, h : h + 1],
                            scale=1.0,
                        )

                # pass 2: y[t_tile, :] = hT[:, t_tile].T @ W2 (+ b2), then LN.
                # LN chain: bias-add + stats on DVE; rsqrt and the final
                # (y-mean)*rstd apply on the ACT engine (func(in*scale+bias)
                # with per-partition scale=rstd, bias=-mean*rstd), so the two
                # engines pipeline across t-tiles and the post-last-matmul
                # tail is short. Output is written fp16 (host upcasts; LN
                # output is O(1) so fp16 rounding ~5e-4 abs, well under the
                # 2e-2 gate) which also halves output DMA bytes.
                for t in range(MT):
                    ps2 = pp.tile([P, D_OUT], f32, tag="ps")
                    for k in range(KH):
                        w2_rhs = (
                            w2_t[k // 2][:, k % 2, :]
                            if isinstance(w2_t, tuple)
                            else w2_t[:, k, :]
                        )
                        nc.tensor.matmul(
                            ps2,
                            lhsT=hT_t[:, k, P * t : P * (t + 1)],
                            rhs=w2_rhs,
                            start=(k == 0),
                            stop=(k == KH - 1),
                        )
                    # y held in fp16: bn_stats and the LN apply then run at
                    # the DVE's 2x 16-bit rate; fp16 rounding of pre-LN y is
                    # ~5e-4 relative, far under the 2e-2 gate
                    y_t = yp.tile([P, D_OUT], store_dt, tag="y")
                    nc.vector.tensor_add(out=y_t, in0=ps2, in1=b2_t[:, 0, :])
                    stats = st.tile([P, 6], f32, tag="stats")
                    nc.vector.bn_stats(out=stats, in_=y_t)
                    mv = st.tile([P, 2], f32, tag="mv")
                    nc.vector.bn_aggr(out=mv, in_=stats)
                    rstd = st.tile([P, 1], f32, tag="rstd")
                    nc.scalar.activation(
                        out=rstd,
                        in_=mv[:, 1:2],
                        func=mybir.ActivationFunctionType.Sqrt,
                        bias=eps_t,
                        scale=1.0,
                    )
                    nc.vector.reciprocal(out=rstd, in_=rstd)
                    y16 = yp.tile([P, D_OUT], store_dt, tag="y16")
                    if apply_affine:
                        ya = yp.tile([P, D_OUT], f32, tag="ya")
                        nc.vector.tensor_scalar(
                            out=ya,
                            in0=y_t,
                            scalar1=mv[:, 0:1],
                            scalar2=rstd,
                            op0=mybir.AluOpType.subtract,
                            op1=mybir.AluOpType.mult,
                        )
                        nc.vector.tensor_mul(out=ya, in0=ya, in1=gm_t[:, 0, :])
                        nc.vector.tensor_add(out=y16, in0=ya, in1=bt_t[:, 0, :])
                    else:
                        nc.vector.tensor_scalar(
                            out=y16,
                            in0=y_t,
                            scalar1=mv[:, 0:1],
                            scalar2=rstd,
                            op0=mybir.AluOpType.subtract,
                            op1=mybir.AluOpType.mult,
                        )
                    nc.sync.dma_start(out=y_d[s, P * t : P * (t + 1), :], in_=y16)
    nc.finalize()
    return nc


def kernel(**inputs) -> np.ndarray:
    global last_run_result
    x = np.asarray(inputs["x"], dtype=np.float32)
    day = np.asarray(inputs["day_indices"]).astype(np.int64)
    W1 = np.asarray(inputs["W1"], dtype=np.float32)
    b1 = np.asarray(inputs["b1"], dtype=np.float32)
    W2 = np.asarray(inputs["W2"], dtype=np.float32)
    b2 = np.asarray(inputs["b2"], dtype=np.float32)
    gamma = np.asarray(inputs["gamma"], dtype=np.float32)
    beta = np.asarray(inputs["beta"], dtype=np.float32)

    apply_affine = not (np.all(gamma == 1.0) and np.all(beta == 0.0))
    key = (MM_DTYPE, apply_affine)
    if key not in _cache:
        _cache[key] = _build(*key)
    nc = _cache[key]

    mm_np = {
        "bfloat16": ml_dtypes.bfloat16,
        "float16": np.float16,
    }.get(MM_DTYPE, np.float32)

    # host-side routing gather + layout prep: K on partitions, and
    # partition-major so each partition's DMA data is contiguous in DRAM
    xt = np.ascontiguousarray(
        x.transpose(0, 2, 1).reshape(B, KD, P, T).transpose(0, 2, 1, 3).astype(mm_np)
    )
    # [B, P, half, KD, D_HID//2]: half-major so pass-1 h0-3 gates on one DMA
    W1d = np.ascontiguousarray(
        W1[day]
        .reshape(B, KD, P, 2, D_HID // 2)
        .transpose(0, 2, 3, 1, 4)
        .astype(mm_np)
    )
    W2d = np.ascontiguousarray(
        W2[day].reshape(B, KH, P, D_OUT).transpose(0, 2, 1, 3).astype(mm_np)
    )
    b1d = np.ascontiguousarray(b1[day].reshape(B, KH, P).transpose(0, 2, 1))
    b2d = np.ascontiguousarray(b2[day])
    gmd = np.ascontiguousarray(gamma[day])
    btd = np.ascontiguousarray(beta[day])

    in_maps = []
    for c in range(N_CORES):
        sl = slice(c * S, (c + 1) * S)
        m = {
            "xt": xt[sl],
            "w1": W1d[sl],
            "b1": b1d[sl],
            "w2": W2d[sl],
            "b2": b2d[sl],
        }
        if apply_affine:
            m["gm"] = gmd[sl]
            m["bt"] = btd[sl]
        in_maps.append(m)

    trace = os.environ.get("DAYMLP_TRACE", "0") == "1"
    res = run_bass_kernel_spmd(
        nc,
        in_maps,
        core_ids=list(range(N_CORES)),
        trace=trace,
    )
    last_run_result = res
    y = np.concatenate([np.asarray(r["y"], dtype=np.float32) for r in res.results], axis=0)
    return y

